# revision 1
# baseline (speedup 1.0000x reference)
"""Temporal attention kernel for Trainium2, data-parallel over batch on 8 cores.

Reference computation (B=64, T=256, D=128, H=8, E=128):
    Q = x@Wq + bq; K = x@Wk + bk; V = x@Wv + bv          [B,T,H,E]
    scores  = einsum('bthd,bjhd->bhtj', Q, K)            [B,H,T,T]
    summary = (scale*scores) @ Ws + bs                   [B,H,T,1]
    beta    = softmax(summary, axis=t)                   [B,H,T]
    result  = sum_t V[b,t,h,:] * beta[b,h,t]             [B,H,E]
    out     = result.reshape(B,H*E) @ Wo + bo            [B,D]

Algebraic restructure (exact up to fp reassociation):
  * Ws contracts the key axis j immediately, so K enters only through
      Ks[b,:] = (Ws^T x_b) @ Wk + sum(Ws)*bk             [HE]
    and Q enters only through per-head dots with Ks:
      summary[t,h] = x_b[t,:] @ (scale*Wq[:,hE:hE+E] @ Ks[hE:hE+E])
  * softmax over t is shift-invariant => the bq/bs bias terms (constant in t)
    drop out entirely.  Logits are O(0.05), so exp() without max-subtraction
    is exact; normalization is deferred to the tiny [d,(b,h)] aggregate.
  * V enters only through sum_t beta[t,h] x_b[t,:], since sum_t beta = 1:
      result[h,:] = (beta^T x_b)[h,:] @ Wv[:,hE:] + bv[hE:]
  This removes the [B,H,T,T] scores tensor and all three full projections:
  ~13 GFLOP -> ~140 MFLOP, leaving the kernel DMA-bound (~3MB/core).

All matmuls are oriented so N (moving free dim) stays small (8) and outputs
land in the layout the next stage consumes - the only transposes are x->xT
(16) and the two tiny [128,64] flips around softmax normalization.
"""

import contextlib

import numpy as np

import concourse.bacc as bacc
import concourse.bass as bass
import concourse.mybir as mybir
import concourse.tile as tile
from concourse.bass_utils import run_bass_kernel_spmd

N_CORES = 8
B, T, D = 64, 256, 128
H, E = 8, 128
HE = H * E
BL = B // N_CORES          # samples per core
TC = T // 128              # 128-token chunks per sample (2)
NJ = BL * TC               # token chunks per core (16)
SCALE = 1.0 / float(np.sqrt(np.float32(E)))

FP32 = mybir.dt.float32
AF = mybir.ActivationFunctionType

# consts blob column layout: [ident | ws | bot | bkT | bvT]
C_ID, C_WS, C_BOT, C_BKT, C_BVT = 0, 128, 130, 131, 139
C_TOT = 147

_cached = {}


def _build_program():
    nc = bacc.Bacc("TRN2", target_bir_lowering=False, debug=False)

    x_d = nc.dram_tensor("x", [BL, T, D], FP32, kind="ExternalInput").ap()
    cst_d = nc.dram_tensor("cst", [128, C_TOT], FP32, kind="ExternalInput").ap()
    wk_d = nc.dram_tensor("wk", [D, HE], FP32, kind="ExternalInput").ap()
    wqt_d = nc.dram_tensor("wqt", [HE, D], FP32, kind="ExternalInput").ap()
    wv_d = nc.dram_tensor("wv", [D, HE], FP32, kind="ExternalInput").ap()
    wor_d = nc.dram_tensor("wor", [D, HE], FP32, kind="ExternalInput").ap()
    y_d = nc.dram_tensor("y", [BL, D], FP32, kind="ExternalOutput").ap()

    with tile.TileContext(nc) as tc:
        _emit(tc, x_d, cst_d, wk_d, wqt_d, wv_d, wor_d, y_d)
    nc.compile()
    return nc


def _emit(tc, x_d, cst_d, wk_d, wqt_d, wv_d, wor_d, y_d):
    nc = tc.nc
    with contextlib.ExitStack() as ctx:
        cpool = ctx.enter_context(tc.tile_pool(name="consts", bufs=1))
        ppool = ctx.enter_context(tc.tile_pool(name="psums", bufs=1,
                                               space="PSUM"))

        # ---- persistent SBUF tiles ----
        cst = cpool.tile([128, C_TOT], FP32, tag="cst")
        x_sb = cpool.tile([128, NJ, D], FP32, tag="x")      # [t, (b,c), d]
        xt_sb = cpool.tile([128, NJ, 128], FP32, tag="xt")  # [d, (b,c), t]
        wk_sb = cpool.tile([128, HE], FP32, tag="wk")       # [d, he]
        wqt_sb = cpool.tile([128, H, D], FP32, tag="wqt")   # [e, h, d]
        wv_sb = cpool.tile([128, HE], FP32, tag="wv")       # [d, he]
        wor_sb = cpool.tile([128, HE], FP32, tag="wor")     # wo as [k,(h,d)]

        sws_sb = cpool.tile([128, 1], FP32, tag="sws")      # sum(Ws) bcast
        bkw_sb = cpool.tile([128, H], FP32, tag="bkw")      # bkT * sum(Ws)
        xst_sb = cpool.tile([128, BL], FP32, tag="xst")     # [d, b]
        kst_sb = cpool.tile([128, H, BL], FP32, tag="kst")  # [e, h, b]
        wqh_sb = cpool.tile([128, H, BL], FP32, tag="wqh")  # [d, h, b]
        e_sb = cpool.tile([128, TC, BL, H], FP32, tag="esb")  # [t, c, b, h]
        xbtu_sb = cpool.tile([128, BL, H], FP32, tag="xbtu")  # [d, b, h]
        xbtn_sb = cpool.tile([64, 128], FP32, tag="xbtn")   # [(b,h), d]
        xbt_sb = cpool.tile([128, BL, H], FP32, tag="xbt")  # [d, b, h]
        rec_sb = cpool.tile([64, 1], FP32, tag="rec")       # 1/esum (b,h)
        rest_sb = cpool.tile([128, H, BL], FP32, tag="rest")  # [e, h, b]
        outt_sb = cpool.tile([128, BL], FP32, tag="outt")   # [dout, b]
        y_sb = cpool.tile([BL, D], FP32, tag="ysb")

        ones_sb = cpool.tile([128, 128], FP32, tag="ones")
        ident = cst[:, C_ID:C_ID + 128]
        ones128 = ones_sb[:]
        bot = cst[:, C_BOT:C_BOT + 1]
        bkt = cst[:, C_BKT:C_BKT + H]
        bvt = cst[:, C_BVT:C_BVT + H]

        # ---- input DMAs, in dependency-criticality order ----
        nc.sync.dma_start(cst[:], cst_d)
        nc.vector.memset(ones_sb[:], 1.0)
        xr = x_d.rearrange("b (c t) d -> t (b c) d", t=128)
        for s in range(4):
            nc.sync.dma_start(x_sb[:, s * NJ // 4:(s + 1) * NJ // 4, :],
                              xr[:, s * NJ // 4:(s + 1) * NJ // 4, :])
        nc.sync.dma_start(wk_sb[:], wk_d)
        nc.sync.dma_start(wqt_sb[:], wqt_d.rearrange("(h e) d -> e h d", e=128))
        nc.sync.dma_start(wv_sb[:], wv_d)
        nc.sync.dma_start(wor_sb[:], wor_d)

        # ---- sum(Ws) broadcast down all partitions, then bkw = bkT*sws ----
        sws_ps = ppool.tile([128, 1], FP32, tag="mm8", bufs=1)
        for c in range(TC):
            nc.tensor.matmul(sws_ps[:], ones128, cst[:, C_WS + c:C_WS + c + 1],
                             start=(c == 0), stop=(c == TC - 1))
        nc.vector.tensor_copy(sws_sb[:], sws_ps[:])
        nc.vector.tensor_scalar_mul(bkw_sb[:], bkt, sws_sb[:])

        # ---- xsT[d, b] = sum_t Ws[t] x_b[t, d] ----
        xst_ps = ppool.tile([128, BL], FP32, tag="mm8", bufs=1)
        for b in range(BL):
            for c in range(TC):
                nc.tensor.matmul(xst_ps[:, b:b + 1], x_sb[:, b * TC + c, :],
                                 cst[:, C_WS + c:C_WS + c + 1],
                                 start=(c == 0), stop=(c == TC - 1))
        nc.vector.tensor_copy(xst_sb[:], xst_ps[:])

        # ---- KsT[e, h, b] = Wk_h^T xs + sum(Ws)*bk_h (rank-1 accumulate) ----
        kst_ps = ppool.tile([128, H, BL], FP32, tag="hb64", bufs=1)
        for h in range(H):
            nc.tensor.matmul(kst_ps[:, h, :], wk_sb[:, h * E:(h + 1) * E],
                             xst_sb[:], start=True, stop=True)
        nc.vector.tensor_add(kst_sb[:], kst_ps[:],
                             bkw_sb[:, :, None].broadcast_to([128, H, BL]))

        # ---- WqhT[d, h, b] (scale pre-folded into wqt on host) ----
        wqh_ps = ppool.tile([128, H, BL], FP32, tag="hb64", bufs=1)
        for h in range(H):
            nc.tensor.matmul(wqh_ps[:, h, :], wqt_sb[:, h, :], kst_sb[:, h, :],
                             start=True, stop=True)
        nc.vector.tensor_copy(wqh_sb[:], wqh_ps[:])

        # ---- xT: transpose x chunks, 4 per PSUM bank ----
        for p in range(NJ // 4):
            tp = ppool.tile([128, 512], FP32, tag="tpx", bufs=2)
            for q in range(4):
                nc.tensor.transpose(tp[:, q * 128:(q + 1) * 128],
                                    x_sb[:, 4 * p + q, :], ident)
            if p % 2 == 0:
                nc.vector.tensor_copy(xt_sb[:, 4 * p:4 * p + 4, :], tp[:])
            else:
                nc.scalar.copy(xt_sb[:, 4 * p:4 * p + 4, :], tp[:])

        # ---- summary[t, c, b, h] then E = exp(summary) in one shot ----
        summ_ps = ppool.tile([128, TC, BL, H], FP32, tag="summ", bufs=1)
        for b in range(BL):
            for c in range(TC):
                j = b * TC + c
                nc.tensor.matmul(summ_ps[:, c, b, :], xt_sb[:, j, :],
                                 wqh_sb[:, :, b], start=True, stop=True)
        nc.scalar.activation(e_sb[:], summ_ps[:], AF.Exp)

        # ---- esum[(b,h)] via ones-matmul over t, both chunks ----
        esum_ps = ppool.tile([64, 1], FP32, tag="mm8", bufs=1)
        for c in range(TC):
            nc.tensor.matmul(esum_ps[:], e_sb[:, c], ones128[:, :1],
                             start=(c == 0), stop=(c == TC - 1))
        nc.vector.reciprocal(rec_sb[:], esum_ps[:])

        # ---- xbtU[d, b, h] = sum_t x[t,d] E[t,(b,c),h] ----
        xbtu_ps = ppool.tile([128, BL, H], FP32, tag="xbtu", bufs=1)
        for b in range(BL):
            for c in range(TC):
                j = b * TC + c
                nc.tensor.matmul(xbtu_ps[:, b, :], x_sb[:, j, :],
                                 e_sb[:, c, b, :],
                                 start=(c == 0), stop=(c == TC - 1))
        nc.vector.tensor_copy(xbtu_sb[:], xbtu_ps[:])

        # ---- normalize: transpose, scale rows by 1/esum, transpose back ----
        xbtn_ps = ppool.tile([64, 128], FP32, tag="mm8", bufs=1)
        nc.tensor.transpose(xbtn_ps[:], xbtu_sb.rearrange("d b h -> d (b h)"),
                            ident)
        nc.scalar.activation(xbtn_sb[:], xbtn_ps[:], AF.Copy, scale=rec_sb[:])
        xbt_ps = ppool.tile([128, 64], FP32, tag="mm8", bufs=1)
        nc.tensor.transpose(xbt_ps[:], xbtn_sb[:], ident[:64, :64])
        nc.vector.tensor_copy(xbt_sb.rearrange("d b h -> d (b h)"), xbt_ps[:])

        # ---- resultT[e, h, b] = Wv_h^T xbt[:, :, h] + bv_h (rank-1) ----
        rest_ps = ppool.tile([128, H, BL], FP32, tag="hb64", bufs=1)
        for h in range(H):
            nc.tensor.matmul(rest_ps[:, h, :], wv_sb[:, h * E:(h + 1) * E],
                             xbt_sb[:, :, h], start=True, stop=True)
        nc.vector.tensor_add(rest_sb[:], rest_ps[:],
                             bvt[:, :, None].broadcast_to([128, H, BL]))

        # ---- outT[dout, b] = sum_h Wo_h^T restT[:, h, :], + bo ----
        outt_ps = ppool.tile([128, BL], FP32, tag="outt", bufs=1)
        for h in range(H):
            nc.tensor.matmul(outt_ps[:], wor_sb[:, h * E:(h + 1) * E],
                             rest_sb[:, h, :], start=(h == 0), stop=(h == H - 1))
        nc.scalar.activation(outt_sb[:], outt_ps[:], AF.Identity, bias=bot)

        # ---- y[b, dout]: store via transposing DMA access pattern ----
        nc.sync.dma_start(y_d.rearrange("b d -> d b"), outt_sb[:])


def _prep_in_maps(inputs):
    x = np.ascontiguousarray(inputs["x"], dtype=np.float32)
    Wq = np.asarray(inputs["Wq"], dtype=np.float32)
    Wv = np.asarray(inputs["Wv"], dtype=np.float32)
    Wo = np.asarray(inputs["Wo"], dtype=np.float32)
    Ws = np.asarray(inputs["Ws"], dtype=np.float32).reshape(T)

    cst = np.zeros((128, C_TOT), dtype=np.float32)
    cst[:, C_ID:C_ID + 128] = np.eye(128, dtype=np.float32)
    for c in range(TC):
        cst[:, C_WS + c] = Ws[c * 128:(c + 1) * 128]
    cst[:, C_BOT] = np.asarray(inputs["bo"], dtype=np.float32)
    cst[:, C_BKT:C_BKT + H] = (
        np.asarray(inputs["bk"], dtype=np.float32).reshape(H, E).T)
    cst[:, C_BVT:C_BVT + H] = (
        np.asarray(inputs["bv"], dtype=np.float32).reshape(H, E).T)

    wo_r = Wo.reshape(H, E, D).transpose(1, 0, 2).reshape(E, H * D)
    shared = {
        "cst": cst,
        "wk": np.ascontiguousarray(inputs["Wk"], dtype=np.float32),
        "wqt": np.ascontiguousarray((SCALE * Wq).T),
        "wv": np.ascontiguousarray(Wv),
        "wor": np.ascontiguousarray(wo_r),
    }
    return [
        {"x": np.ascontiguousarray(x[c * BL:(c + 1) * BL]), **shared}
        for c in range(N_CORES)
    ]


def kernel(**inputs):
    if "nc" not in _cached:
        _cached["nc"] = _build_program()
    nc = _cached["nc"]
    in_maps = _prep_in_maps(inputs)
    res = run_bass_kernel_spmd(nc, in_maps, list(range(N_CORES)))
    _cached["last_results"] = res
    return np.concatenate([res.results[c]["y"] for c in range(N_CORES)], axis=0)



# revision 11
# speedup vs baseline: 1.1593x; 1.1593x over previous
"""Temporal attention kernel for Trainium2, data-parallel over batch on 8 cores.

Reference computation (B=64, T=256, D=128, H=8, E=128):
    Q = x@Wq + bq; K = x@Wk + bk; V = x@Wv + bv          [B,T,H,E]
    scores  = einsum('bthd,bjhd->bhtj', Q, K)            [B,H,T,T]
    summary = (scale*scores) @ Ws + bs                   [B,H,T,1]
    beta    = softmax(summary, axis=t)                   [B,H,T]
    result  = sum_t V[b,t,h,:] * beta[b,h,t]             [B,H,E]
    out     = result.reshape(B,H*E) @ Wo + bo            [B,D]

Algebraic restructure (exact up to fp reassociation):
  * Ws contracts the key axis immediately, so K enters only through
    xs_b = x_b^T Ws, and the logits are
      summary[t,h] = x_b[t,:] @ (M_h xs_b + sws*q0_h)
    with M_h = scale*Wq_h Wk_h^T and q0_h = scale*Wq_h bk_h folded on host.
  * softmax over t is shift-invariant => bq/bs terms drop; logits are O(0.05)
    so exp() without max-subtraction is exact.
  * V and the output projection collapse: out = sum_h N_h^T xbt_h + b0 with
    N_h = Wv_h Wo_h and b0 = bo + Wo^T bv folded on host (b0 added on host).
  All device weights travel as bf16 (M is logits-only; N/x bf16 keeps rel err
  ~3e-3 << 2e-2 tol): per-core DMA is ~1.1 MB vs 3 MB for the fp32 version.
"""

import contextlib

import numpy as np

import concourse.bacc as bacc
import concourse.bass as bass
import concourse.mybir as mybir
import concourse.tile as tile
from concourse.bass_utils import run_bass_kernel_spmd

N_CORES = 8
B, T, D = 64, 256, 128
H, E = 8, 128
HE = H * E
BL = B // N_CORES          # samples per core (8)
TC = T // 128              # 128-token chunks per sample (2)
NJ = BL * TC               # token chunks per core (16)
NG = 2                     # sample groups per core (pipelining)
GB = BL // NG              # samples per group (4)
SCALE = 1.0 / float(np.sqrt(np.float32(E)))

FP32 = mybir.dt.float32
BF16 = mybir.dt.bfloat16
NP_BF16 = mybir.dt.np(BF16)
AF = mybir.ActivationFunctionType

# const blob column layout (all bf16): [ident | ws | q0s | ones | MT | NN]
C_ID = 0
C_WS = C_ID + 128          # Ws chunks, TC cols
C_Q0 = C_WS + TC           # sws*scale*Wq_h@bk_h, H cols
C_ONE = C_Q0 + H           # ones column
C_MT = C_ONE + 1           # scale*Wk_h@Wq_h^T, H*128 cols
C_NN = C_MT + H * 128      # Wv_h@Wo_h, H*128 cols
C_TOT = C_NN + H * 128
CA_TOT = C_MT + H * 128    # first blob: everything through MT

_cached = {}


def _build_program():
    nc = bacc.Bacc("TRN2", target_bir_lowering=False, debug=False)

    wba_d = nc.dram_tensor("wba", [128, CA_TOT], BF16, kind="ExternalInput").ap()
    wbn_d = nc.dram_tensor("wbn", [128, H * 128], BF16, kind="ExternalInput").ap()
    x_d = nc.dram_tensor("xb", [128, NJ, 128], BF16, kind="ExternalInput").ap()
    y_d = nc.dram_tensor("y", [D, BL], FP32, kind="ExternalOutput").ap()

    with tile.TileContext(nc) as tc:
        _emit(tc, wba_d, wbn_d, x_d, y_d)
    nc.compile()
    return nc


def _emit(tc, wba_d, wbn_d, x_d, y_d):
    nc = tc.nc
    with contextlib.ExitStack() as ctx:
        cpool = ctx.enter_context(tc.tile_pool(name="consts", bufs=1))
        ppool = ctx.enter_context(tc.tile_pool(name="psums", bufs=1,
                                               space="PSUM"))

        # ---- persistent SBUF tiles ----
        wb = cpool.tile([128, C_TOT], BF16, tag="wb")
        x_sb = cpool.tile([128, NJ, 128], BF16, tag="x")    # [t, (b,c), d]
        xt_sb = cpool.tile([128, NJ, 128], BF16, tag="xt")  # [d, (b,c), t]
        xst_sb = cpool.tile([128, BL], BF16, tag="xst")     # [d, b]
        q0f_sb = cpool.tile([128, H], FP32, tag="q0f")      # q0s upcast
        wqh_sb = cpool.tile([128, H, BL], BF16, tag="wqh")  # [d, h, b]
        e_sb = cpool.tile([128, TC, BL, H], BF16, tag="esb")  # [t, c, b, h]
        rec_sb = [cpool.tile([GB * H, 1], FP32, tag=f"rec{g}", name=f"rec{g}")
                  for g in range(NG)]                       # 1/esum (b,h)
        xbtu_sb = cpool.tile([128, BL, H], BF16, tag="xbtu")  # [d, (b,h)]
        xbtn_sb = [cpool.tile([GB * H, 128], BF16, tag=f"xbtn{g}", name=f"xbtn{g}")
                   for g in range(NG)]                      # [(b,h), d]
        xbt_sb = cpool.tile([128, BL, H], BF16, tag="xbt")  # [d, b, h]
        outt_sb = cpool.tile([128, BL], FP32, tag="outt")   # [dout, b]

        ident = wb[:, C_ID:C_ID + 128]
        ones_col = wb[:, C_ONE:C_ONE + 1]

        # ---- input DMAs: wbA, x group0, x group1, wbN ----
        nc.sync.dma_start(wb[:, :CA_TOT], wba_d)
        HJ = NJ // NG
        nc.sync.dma_start(x_sb[:, :HJ, :], x_d[:, :HJ, :])
        nc.sync.dma_start(x_sb[:, HJ:, :], x_d[:, HJ:, :])
        nc.sync.dma_start(wb[:, C_NN:], wbn_d)

        # q0s upcast to fp32 off the critical path
        nc.vector.tensor_copy(q0f_sb[:], wb[:, C_Q0:C_Q0 + H])

        # ---- shared PSUM tiles; the two sample groups use disjoint
        # ---- columns/partitions so they can overlap in time ----
        ps_xe = ppool.tile([128, 16], FP32, tag="xe")    # xst cols 0-7, esum 8-9
        ps_wqh = ppool.tile([128, H, BL], FP32, tag="wqh")
        ps_summ = ppool.tile([128, TC, BL, H], FP32, tag="summ")
        ps_xbtu = ppool.tile([128, BL, H], FP32, tag="xbtu")
        ps_nrm = ppool.tile([128, 512 + NG * GB * H], BF16, tag="nrm")
        ps_out = ppool.tile([128, BL], FP32, tag="outp")

        for g in range(NG):
            b0, b1 = g * GB, (g + 1) * GB
            j0 = b0 * TC

            # ---- xT: transpose this group's x chunks, 4 per PSUM tile ----
            for p in range(2):
                ps_tp = ppool.tile([128, 512], BF16, tag="tpx", bufs=2,
                                   name=f"tp{g}{p}")
                lo = j0 + 4 * p
                for q in range(4):
                    nc.tensor.transpose(ps_tp[:, q * 128:(q + 1) * 128],
                                        x_sb[:, lo + q, :], ident)
                if p == 0:
                    nc.vector.tensor_copy(xt_sb[:, lo:lo + 4, :], ps_tp[:])
                else:
                    nc.scalar.copy(xt_sb[:, lo:lo + 4, :], ps_tp[:])

            # ---- xsT[d, b] = sum_t Ws[t] x_b[t, d] ----
            for b in range(b0, b1):
                for c in range(TC):
                    nc.tensor.matmul(ps_xe[:, b:b + 1],
                                     x_sb[:, b * TC + c, :],
                                     wb[:, C_WS + c:C_WS + c + 1],
                                     start=(c == 0), stop=(c == TC - 1))
            nc.vector.tensor_copy(xst_sb[:, b0:b1], ps_xe[:, b0:b1])

            # ---- wqh[d, h, b] = M_h^T.T @ xst + sws*q0_h ----
            for h in range(H):
                nc.tensor.matmul(ps_wqh[:, h, b0:b1],
                                 wb[:, C_MT + h * 128:C_MT + (h + 1) * 128],
                                 xst_sb[:, b0:b1], start=True, stop=True)
            nc.vector.tensor_add(
                wqh_sb[:, :, b0:b1], ps_wqh[:, :, b0:b1],
                q0f_sb[:, :, None].broadcast_to([128, H, GB]))

            # ---- summary[t, c, b, h] then E = exp(summary) ----
            for b in range(b0, b1):
                for c in range(TC):
                    j = b * TC + c
                    nc.tensor.matmul(ps_summ[:, c, b, :],
                                     xt_sb[:, j, :], wqh_sb[:, :, b],
                                     start=True, stop=True)
            nc.scalar.activation(e_sb[:, :, b0:b1, :], ps_summ[:, :, b0:b1, :],
                                 AF.Exp)

            # ---- esum[(b,h)] via ones-matmul over t, both chunks ----
            for c in range(TC):
                nc.tensor.matmul(ps_xe[:GB * H, 8 + g:9 + g],
                                 e_sb[:, c, b0:b1, :], ones_col,
                                 start=(c == 0), stop=(c == TC - 1))
            nc.vector.reciprocal(rec_sb[g][:], ps_xe[:GB * H, 8 + g:9 + g])

            # ---- xbtU[d, b, h] = sum_t x[t,d] E[t,(b,c),h] ----
            for b in range(b0, b1):
                for c in range(TC):
                    j = b * TC + c
                    nc.tensor.matmul(ps_xbtu[:, b, :], x_sb[:, j, :],
                                     e_sb[:, c, b, :],
                                     start=(c == 0), stop=(c == TC - 1))
            nc.vector.tensor_copy(xbtu_sb[:, b0:b1, :], ps_xbtu[:, b0:b1, :])

            # ---- normalize: transpose, scale rows by 1/esum, transpose ----
            xbtn_ps = ps_nrm[:GB * H, g * 256:g * 256 + 128]
            nc.tensor.transpose(
                xbtn_ps,
                xbtu_sb[:, b0:b1, :].rearrange("d b h -> d (b h)"),
                ident)
            nc.scalar.activation(xbtn_sb[g][:], xbtn_ps, AF.Copy,
                                 scale=rec_sb[g][:])
            xbt_ps = ps_nrm[:, 512 + g * GB * H:512 + (g + 1) * GB * H]
            nc.tensor.transpose(xbt_ps, xbtn_sb[g][:],
                                ident[:GB * H, :GB * H])
            nc.vector.tensor_copy(
                xbt_sb[:, b0:b1, :].rearrange("d b h -> d (b h)"), xbt_ps)

            # ---- outT[dout, b] = sum_h N_h^T xbt[:, b, h] ----
            for h in range(H):
                nc.tensor.matmul(ps_out[:, b0:b1],
                                 wb[:, C_NN + h * 128:C_NN + (h + 1) * 128],
                                 xbt_sb[:, b0:b1, h],
                                 start=(h == 0), stop=(h == H - 1))
            nc.scalar.copy(outt_sb[:, b0:b1], ps_out[:, b0:b1])

        # ---- y[dout, b] in one DMA; host transposes and adds b0 ----
        nc.sync.dma_start(y_d, outt_sb[:])


def _prep_in_maps(inputs):
    x = np.asarray(inputs["x"], dtype=np.float32)
    Wq = np.asarray(inputs["Wq"], dtype=np.float32)
    Wk = np.asarray(inputs["Wk"], dtype=np.float32)
    Wv = np.asarray(inputs["Wv"], dtype=np.float32)
    Wo = np.asarray(inputs["Wo"], dtype=np.float32)
    bk = np.asarray(inputs["bk"], dtype=np.float32)
    Ws = np.asarray(inputs["Ws"], dtype=np.float32).reshape(T)
    sws = float(Ws.sum())

    wb = np.zeros((128, C_TOT), dtype=np.float32)
    wb[:, C_ID:C_ID + 128] = np.eye(128, dtype=np.float32)
    for c in range(TC):
        wb[:, C_WS + c] = Ws[c * 128:(c + 1) * 128]
    wb[:, C_ONE] = 1.0
    for h in range(H):
        Wq_h = Wq[:, h * E:(h + 1) * E]
        Wk_h = Wk[:, h * E:(h + 1) * E]
        Wv_h = Wv[:, h * E:(h + 1) * E]
        Wo_h = Wo[h * E:(h + 1) * E, :]
        wb[:, C_Q0 + h] = sws * SCALE * (Wq_h @ bk[h * E:(h + 1) * E])
        wb[:, C_MT + h * 128:C_MT + (h + 1) * 128] = SCALE * (Wk_h @ Wq_h.T)
        wb[:, C_NN + h * 128:C_NN + (h + 1) * 128] = Wv_h @ Wo_h
    wb = wb.astype(NP_BF16)

    shared = {
        "wba": np.ascontiguousarray(wb[:, :CA_TOT]),
        "wbn": np.ascontiguousarray(wb[:, C_NN:]),
    }
    in_maps = []
    for core in range(N_CORES):
        xc = x[core * BL:(core + 1) * BL]                  # [BL, T, D]
        xr = xc.reshape(BL, TC, 128, D).transpose(2, 0, 1, 3)
        xr = np.ascontiguousarray(xr.astype(NP_BF16)).reshape(128, NJ, D)
        in_maps.append({"xb": xr, **shared})
    return in_maps


def kernel(**inputs):
    if "nc" not in _cached:
        _cached["nc"] = _build_program()
    nc = _cached["nc"]
    in_maps = _prep_in_maps(inputs)
    res = run_bass_kernel_spmd(nc, in_maps, list(range(N_CORES)))
    _cached["last_results"] = res

    Wo = np.asarray(inputs["Wo"], dtype=np.float32)
    bv = np.asarray(inputs["bv"], dtype=np.float32)
    bo = np.asarray(inputs["bo"], dtype=np.float32)
    b0 = bo + bv @ Wo
    return np.concatenate(
        [res.results[c]["y"].T + b0 for c in range(N_CORES)], axis=0
    ).astype(np.float32)


# revision 13
# speedup vs baseline: 1.4077x; 1.2143x over previous
"""Temporal attention kernel for Trainium2, data-parallel over batch on 8 cores.

Reference computation (B=64, T=256, D=128, H=8, E=128):
    Q = x@Wq + bq; K = x@Wk + bk; V = x@Wv + bv          [B,T,H,E]
    scores  = einsum('bthd,bjhd->bhtj', Q, K)            [B,H,T,T]
    summary = (scale*scores) @ Ws + bs                   [B,H,T,1]
    beta    = softmax(summary, axis=t)                   [B,H,T]
    result  = sum_t V[b,t,h,:] * beta[b,h,t]             [B,H,E]
    out     = result.reshape(B,H*E) @ Wo + bo            [B,D]

Algebraic restructure (exact up to fp reassociation):
  * Ws contracts the key axis immediately, so K enters only through
    xs_b = x_b^T Ws, and the logits are
      summary[t,h] = x_b[t,:] @ (M_h xs_b + sws*q0_h)
    with M_h = scale*Wq_h Wk_h^T and q0_h = scale*Wq_h bk_h folded on host.
  * softmax over t is shift-invariant => bq/bs terms drop; logits are O(0.05)
    so exp() without max-subtraction is exact.
  * V and the output projection collapse: out = sum_h N_h^T xbt_h + b0 with
    N_h = Wv_h Wo_h and b0 = bo + Wo^T bv folded on host (b0 added on host).
  All device tensors travel as bf16 (M is logits-only; N/x bf16 keeps rel err
  ~3e-3 << 2e-2 tol): per-core DMA ~1.6 MB split across three DMA queues
  (SP/DVE/Act) so transfers overlap; x ships in both [t,.,d] and [d,.,t]
  layouts so no on-chip transposes of x are needed.
"""

import contextlib

import numpy as np

import concourse.bacc as bacc
import concourse.bass as bass
import concourse.mybir as mybir
import concourse.tile as tile
from concourse.bass_utils import run_bass_kernel_spmd

N_CORES = 8
B, T, D = 64, 256, 128
H, E = 8, 128
HE = H * E
BL = B // N_CORES          # samples per core (8)
TC = T // 128              # 128-token chunks per sample (2)
NJ = BL * TC               # token chunks per core (16)
NG = 2                     # sample groups per core (pipelining)
GB = BL // NG              # samples per group (4)
HJ = NJ // NG              # chunks per group (8)
SCALE = 1.0 / float(np.sqrt(np.float32(E)))

FP32 = mybir.dt.float32
BF16 = mybir.dt.bfloat16
NP_BF16 = mybir.dt.np(BF16)
AF = mybir.ActivationFunctionType

# wbA column layout (bf16): [ident | ws | q0s | ones | MT]
C_ID = 0
C_WS = C_ID + 128          # Ws chunks, TC cols
C_Q0 = C_WS + TC           # sws*scale*Wq_h@bk_h, H cols
C_ONE = C_Q0 + H           # ones column
C_MT = C_ONE + 1           # scale*Wk_h@Wq_h^T, H*128 cols
CA_TOT = C_MT + H * 128

_cached = {}


def _build_program():
    nc = bacc.Bacc("TRN2", target_bir_lowering=False, debug=False)

    wba_d = nc.dram_tensor("wba", [128, CA_TOT], BF16, kind="ExternalInput").ap()
    wbn_d = nc.dram_tensor("wbn", [128, H * 128], BF16, kind="ExternalInput").ap()
    x_d = nc.dram_tensor("xb", [128, NJ, 128], BF16, kind="ExternalInput").ap()
    xt_d = nc.dram_tensor("xtb", [128, NJ, 128], BF16,
                          kind="ExternalInput").ap()
    y_d = nc.dram_tensor("y", [D, BL], FP32, kind="ExternalOutput").ap()

    with tile.TileContext(nc) as tc:
        _emit(tc, wba_d, wbn_d, x_d, xt_d, y_d)
    nc.compile()
    return nc


def _emit(tc, wba_d, wbn_d, x_d, xt_d, y_d):
    nc = tc.nc
    with contextlib.ExitStack() as ctx:
        cpool = ctx.enter_context(tc.tile_pool(name="consts", bufs=1))
        ppool = ctx.enter_context(tc.tile_pool(name="psums", bufs=1,
                                               space="PSUM"))

        # ---- persistent SBUF tiles (x/xT split per group for early sems) ---
        wba = cpool.tile([128, CA_TOT], BF16, tag="wba")
        wbn = cpool.tile([128, H * 128], BF16, tag="wbn")
        x_g = [cpool.tile([128, HJ, 128], BF16, tag=f"x{g}", name=f"x{g}")
               for g in range(NG)]
        xt_g = [cpool.tile([128, HJ, 128], BF16, tag=f"xt{g}", name=f"xt{g}")
                for g in range(NG)]
        xst_sb = cpool.tile([128, BL], BF16, tag="xst")     # [d, b]
        q0f_sb = cpool.tile([128, H], FP32, tag="q0f")      # q0s upcast
        wqh_sb = cpool.tile([128, H, BL], BF16, tag="wqh")  # [d, h, b]
        e_sb = cpool.tile([128, TC, BL, H], BF16, tag="esb")  # [t, c, b, h]
        rec_sb = [cpool.tile([GB * H, 1], FP32, tag=f"rec{g}", name=f"rec{g}")
                  for g in range(NG)]
        xbtu_sb = cpool.tile([128, BL, H], BF16, tag="xbtu")  # [d, (b,h)]
        xbtn_sb = [cpool.tile([GB * H, 128], BF16, tag=f"xbn{g}",
                              name=f"xbn{g}") for g in range(NG)]
        xbt_sb = cpool.tile([128, BL, H], BF16, tag="xbt")  # [d, b, h]
        outt_sb = cpool.tile([128, BL], FP32, tag="outt")   # [dout, b]

        ident = wba[:, C_ID:C_ID + 128]
        ones_col = wba[:, C_ONE:C_ONE + 1]

        # ---- input DMAs spread across three queues ----
        # SP: x group0, x group1;  Pool: xT group0, xT group1, NN;  Act: wbA
        nc.sync.dma_start(x_g[0][:], x_d[:, :HJ, :])
        nc.sync.dma_start(x_g[1][:], x_d[:, HJ:, :])
        nc.gpsimd.dma_start(xt_g[0][:], xt_d[:, :HJ, :])
        nc.gpsimd.dma_start(xt_g[1][:], xt_d[:, HJ:, :])
        nc.gpsimd.dma_start(wbn[:], wbn_d)
        nc.scalar.dma_start(wba[:], wba_d)

        # q0s upcast to fp32 off the critical path
        nc.vector.tensor_copy(q0f_sb[:], wba[:, C_Q0:C_Q0 + H])

        # ---- shared PSUM tiles; groups use disjoint regions ----
        ps_xe = ppool.tile([128, 16], FP32, tag="xe")   # xst cols 0-7, esum 8-9
        ps_wqh = ppool.tile([128, H, BL], FP32, tag="wqh")
        ps_summ = ppool.tile([128, TC, BL, H], FP32, tag="summ")
        ps_xbtu = ppool.tile([128, BL, H], FP32, tag="xbtu")
        ps_nrm = ppool.tile([128, 512 + NG * GB * H], BF16, tag="nrm")
        ps_out = ppool.tile([128, BL], FP32, tag="outp")

        def front(g):
            """xst -> wqh -> summary -> exp for group g."""
            b0, b1 = g * GB, (g + 1) * GB
            xg, xtg = x_g[g], xt_g[g]
            for b in range(b0, b1):
                for c in range(TC):
                    jl = (b - b0) * TC + c
                    nc.tensor.matmul(ps_xe[:, b:b + 1], xg[:, jl, :],
                                     wba[:, C_WS + c:C_WS + c + 1],
                                     start=(c == 0), stop=(c == TC - 1))
            nc.vector.tensor_copy(xst_sb[:, b0:b1], ps_xe[:, b0:b1])
            for h in range(H):
                nc.tensor.matmul(ps_wqh[:, h, b0:b1],
                                 wba[:, C_MT + h * 128:C_MT + (h + 1) * 128],
                                 xst_sb[:, b0:b1], start=True, stop=True)
            nc.vector.tensor_add(
                wqh_sb[:, :, b0:b1], ps_wqh[:, :, b0:b1],
                q0f_sb[:, :, None].broadcast_to([128, H, GB]))
            for b in range(b0, b1):
                for c in range(TC):
                    jl = (b - b0) * TC + c
                    nc.tensor.matmul(ps_summ[:, c, b, :],
                                     xtg[:, jl, :], wqh_sb[:, :, b],
                                     start=True, stop=True)
            nc.scalar.activation(e_sb[:, :, b0:b1, :], ps_summ[:, :, b0:b1, :],
                                 AF.Exp)

        def mid(g):
            """esum -> rec and xbtu -> copy for group g."""
            b0, b1 = g * GB, (g + 1) * GB
            xg = x_g[g]
            for c in range(TC):
                nc.tensor.matmul(ps_xe[:GB * H, 8 + g:9 + g],
                                 e_sb[:, c, b0:b1, :], ones_col,
                                 start=(c == 0), stop=(c == TC - 1))
            nc.vector.reciprocal(rec_sb[g][:], ps_xe[:GB * H, 8 + g:9 + g])
            for b in range(b0, b1):
                for c in range(TC):
                    jl = (b - b0) * TC + c
                    nc.tensor.matmul(ps_xbtu[:, b, :], xg[:, jl, :],
                                     e_sb[:, c, b, :],
                                     start=(c == 0), stop=(c == TC - 1))
            nc.vector.tensor_copy(xbtu_sb[:, b0:b1, :], ps_xbtu[:, b0:b1, :])

        def norm(g):
            """transpose, scale by 1/esum, transpose back for group g."""
            b0, b1 = g * GB, (g + 1) * GB
            xbtn_ps = ps_nrm[:GB * H, g * 256:g * 256 + 128]
            nc.tensor.transpose(
                xbtn_ps,
                xbtu_sb[:, b0:b1, :].rearrange("d b h -> d (b h)"), ident)
            nc.scalar.activation(xbtn_sb[g][:], xbtn_ps, AF.Copy,
                                 scale=rec_sb[g][:])
            xbt_ps = ps_nrm[:, 512 + g * GB * H:512 + (g + 1) * GB * H]
            nc.tensor.transpose(xbt_ps, xbtn_sb[g][:],
                                ident[:GB * H, :GB * H])
            nc.vector.tensor_copy(
                xbt_sb[:, b0:b1, :].rearrange("d b h -> d (b h)"), xbt_ps)

        def back(g):
            """output projection for group g."""
            b0, b1 = g * GB, (g + 1) * GB
            for h in range(H):
                nc.tensor.matmul(ps_out[:, b0:b1],
                                 wbn[:, h * 128:(h + 1) * 128],
                                 xbt_sb[:, b0:b1, h],
                                 start=(h == 0), stop=(h == H - 1))
            nc.scalar.copy(outt_sb[:, b0:b1], ps_out[:, b0:b1])

        # stage-interleaved emission: per-engine program order must never
        # let a later-arriving group's op head-of-line-block an earlier one
        front(0)
        front(1)
        mid(0)
        mid(1)
        norm(0)
        norm(1)
        back(0)
        back(1)

        # ---- y[dout, b] in one DMA; host transposes and adds b0 ----
        nc.sync.dma_start(y_d, outt_sb[:])


def _prep_in_maps(inputs):
    x = np.asarray(inputs["x"], dtype=np.float32)
    Wq = np.asarray(inputs["Wq"], dtype=np.float32)
    Wk = np.asarray(inputs["Wk"], dtype=np.float32)
    Wv = np.asarray(inputs["Wv"], dtype=np.float32)
    Wo = np.asarray(inputs["Wo"], dtype=np.float32)
    bk = np.asarray(inputs["bk"], dtype=np.float32)
    Ws = np.asarray(inputs["Ws"], dtype=np.float32).reshape(T)
    sws = float(Ws.sum())

    wba = np.zeros((128, CA_TOT), dtype=np.float32)
    wba[:, C_ID:C_ID + 128] = np.eye(128, dtype=np.float32)
    for c in range(TC):
        wba[:, C_WS + c] = Ws[c * 128:(c + 1) * 128]
    wba[:, C_ONE] = 1.0
    wbn = np.zeros((128, H * 128), dtype=np.float32)
    for h in range(H):
        Wq_h = Wq[:, h * E:(h + 1) * E]
        Wk_h = Wk[:, h * E:(h + 1) * E]
        Wv_h = Wv[:, h * E:(h + 1) * E]
        Wo_h = Wo[h * E:(h + 1) * E, :]
        wba[:, C_Q0 + h] = sws * SCALE * (Wq_h @ bk[h * E:(h + 1) * E])
        wba[:, C_MT + h * 128:C_MT + (h + 1) * 128] = SCALE * (Wk_h @ Wq_h.T)
        wbn[:, h * 128:(h + 1) * 128] = Wv_h @ Wo_h

    shared = {"wba": wba.astype(NP_BF16), "wbn": wbn.astype(NP_BF16)}
    in_maps = []
    for core in range(N_CORES):
        xc = x[core * BL:(core + 1) * BL]                  # [BL, T, D]
        xq = xc.reshape(BL, TC, 128, D)
        xr = xq.transpose(2, 0, 1, 3)                      # [t, b, c, d]
        xtr = xq.transpose(3, 0, 1, 2)                     # [d, b, c, t]
        in_maps.append({
            "xb": np.ascontiguousarray(xr.astype(NP_BF16)).reshape(
                128, NJ, D),
            "xtb": np.ascontiguousarray(xtr.astype(NP_BF16)).reshape(
                128, NJ, 128),
            **shared,
        })
    return in_maps


def kernel(**inputs):
    if "nc" not in _cached:
        _cached["nc"] = _build_program()
    nc = _cached["nc"]
    in_maps = _prep_in_maps(inputs)
    res = run_bass_kernel_spmd(nc, in_maps, list(range(N_CORES)))
    _cached["last_results"] = res

    Wo = np.asarray(inputs["Wo"], dtype=np.float32)
    bv = np.asarray(inputs["bv"], dtype=np.float32)
    bo = np.asarray(inputs["bo"], dtype=np.float32)
    b0 = bo + bv @ Wo
    return np.concatenate(
        [res.results[c]["y"].T + b0 for c in range(N_CORES)], axis=0
    ).astype(np.float32)


# revision 19
# speedup vs baseline: 1.6832x; 1.1957x over previous
"""Temporal attention kernel for Trainium2, data-parallel over batch on 8 cores.

Reference computation (B=64, T=256, D=128, H=8, E=128):
    Q = x@Wq + bq; K = x@Wk + bk; V = x@Wv + bv          [B,T,H,E]
    scores  = einsum('bthd,bjhd->bhtj', Q, K)            [B,H,T,T]
    summary = (scale*scores) @ Ws + bs                   [B,H,T,1]
    beta    = softmax(summary, axis=t)                   [B,H,T]
    result  = sum_t V[b,t,h,:] * beta[b,h,t]             [B,H,E]
    out     = result.reshape(B,H*E) @ Wo + bo            [B,D]

Algebraic restructure (exact up to fp reassociation):
  * Ws contracts the key axis immediately, so K enters only through
    xs_b = x_b^T Ws, and the logits are
      summary[t,h] = x_b[t,:] @ (M_h xs_b + sws*q0_h)
    with M_h = scale*Wq_h Wk_h^T and q0_h = scale*Wq_h bk_h folded on host.
  * softmax over t is shift-invariant => bq/bs terms drop; logits are O(0.05)
    so exp() without max-subtraction is exact.
  * V and the output projection collapse: out = sum_h N_h^T xbt_h + b0 with
    N_h = Wv_h Wo_h and b0 = bo + Wo^T bv folded on host (b0 added on host).
  * beta normalization: 1/sum(exp) is broadcast down all 128 partitions with
    a rank-1 ones x recRow matmul, so the weighted V-sum is normalized by a
    single elementwise multiply - no transposes anywhere in the kernel.
  Precision: V path (x, N) travels bf16; logits-only tensors (xT, M) travel
  fp8e4m3 (M pre-scaled by 256, compensated in xst) - logits are O(0.05) and
  softmax is shift-tolerant, keeping rel err ~4e-3 << 2e-2 tol.
  x ships in both [t,.,d] (bf16) and [d,.,t] (fp8) layouts so no on-chip
  transposes of x are needed; DMAs are spread over the SP/Pool/Act queues.
"""

import contextlib

import numpy as np

import concourse.bacc as bacc
import concourse.bass as bass
import concourse.mybir as mybir
import concourse.tile as tile
from concourse.bass_utils import run_bass_kernel_spmd

N_CORES = 8
B, T, D = 64, 256, 128
H, E = 8, 128
HE = H * E
BL = B // N_CORES          # samples per core (8)
TC = T // 128              # 128-token chunks per sample (2)
NJ = BL * TC               # token chunks per core (16)
NG = 2                     # sample groups per core (pipelining)
GB = BL // NG              # samples per group (4)
HJ = NJ // NG              # chunks per group (8)
GH = GB * H                # (b,h) pairs per group (32)
SCALE = 1.0 / float(np.sqrt(np.float32(E)))
MSCALE = 256.0             # fp8 underflow guard on M, undone in xst

FP32 = mybir.dt.float32
BF16 = mybir.dt.bfloat16
FP8 = mybir.dt.float8e4
NP_BF16 = mybir.dt.np(BF16)
NP_FP8 = mybir.dt.np(FP8)
AF = mybir.ActivationFunctionType

XW = HJ * 128              # x cols per group
C_WS = XW                  # Ws chunks (TC cols) appended to x group 0
C_Q0 = C_WS + TC           # sws*scale*Wq_h@bk_h, H cols

_cached = {}


def _build_program():
    nc = bacc.Bacc("TRN2", target_bir_lowering=False, debug=False)

    x0_d = nc.dram_tensor("x0", [128, XW + TC + H], BF16,
                          kind="ExternalInput").ap()
    x1_d = nc.dram_tensor("x1", [128, XW], BF16, kind="ExternalInput").ap()
    xt_d = nc.dram_tensor("xt", [128, NJ, 128], FP8, kind="ExternalInput").ap()
    mt_d = nc.dram_tensor("mt", [128, H * 128], FP8, kind="ExternalInput").ap()
    nn_d = nc.dram_tensor("nn", [128, H * 128], BF16, kind="ExternalInput").ap()
    y_d = nc.dram_tensor("y", [D, BL], FP32, kind="ExternalOutput").ap()

    with tile.TileContext(nc) as tc:
        _emit(tc, x0_d, x1_d, xt_d, mt_d, nn_d, y_d)
    nc.compile()
    return nc


def _emit(tc, x0_d, x1_d, xt_d, mt_d, nn_d, y_d):
    nc = tc.nc
    with contextlib.ExitStack() as ctx:
        cpool = ctx.enter_context(tc.tile_pool(name="consts", bufs=1))
        ppool = ctx.enter_context(tc.tile_pool(name="psums", bufs=1,
                                               space="PSUM"))

        # ---- persistent SBUF tiles ----
        x0_sb = cpool.tile([128, XW + TC + H], BF16, tag="x0")
        x1_sb = cpool.tile([128, XW], BF16, tag="x1")
        xt_sb = [cpool.tile([128, HJ, 128], FP8, tag=f"xt{g}", name=f"xt{g}")
                 for g in range(NG)]
        mt_sb = cpool.tile([128, H * 128], FP8, tag="mt")
        nn_sb = cpool.tile([128, H * 128], BF16, tag="nn")
        onec_sb = cpool.tile([128, 1], BF16, tag="onec")    # ones column
        oner_sb = cpool.tile([1, 128], FP32, tag="oner")    # ones row
        c256_sb = cpool.tile([128, 1], FP32, tag="c256")    # 1/MSCALE
        xst_sb = cpool.tile([128, BL], BF16, tag="xst")     # [d, b] (/MSCALE)
        wqh_sb = cpool.tile([128, H, BL], BF16, tag="wqh")  # [d, h, b]
        e_sb = cpool.tile([128, TC, BL, H], BF16, tag="esb")  # [t, c, b, h]
        recr_sb = [cpool.tile([1, GH], FP32, tag=f"rr{g}", name=f"rr{g}")
                   for g in range(NG)]                      # 1/esum row
        xbtu_sb = cpool.tile([128, BL, H], BF16, tag="xbtu")  # unnormalized
        xbt_sb = cpool.tile([128, BL, H], BF16, tag="xbt")  # normalized
        outt_sb = cpool.tile([128, BL], FP32, tag="outt")   # [dout, b]

        x_g = [x0_sb[:, :XW].rearrange("t (j d) -> t j d", d=128),
               x1_sb.rearrange("t (j d) -> t j d", d=128)]
        ws_col = [x0_sb[:, C_WS + c:C_WS + c + 1] for c in range(TC)]
        q0s = x0_sb[:, C_Q0:C_Q0 + H]

        # ---- input DMAs across the three DMA-capable queues ----
        # SP: x0(+ws,q0), MT;  Pool: x1, xT0, xT1;  Act: (act table), NN
        nc.sync.dma_start(x0_sb[:], x0_d)
        nc.sync.dma_start(mt_sb[:], mt_d)
        nc.gpsimd.dma_start(x1_sb[:], x1_d)
        nc.gpsimd.dma_start(xt_sb[0][:], xt_d[:, :HJ, :])
        nc.gpsimd.dma_start(xt_sb[1][:], xt_d[:, HJ:, :])
        nc.scalar.dma_start(nn_sb[:], nn_d)

        # constants built on idle DVE lanes at t~0
        nc.vector.memset(onec_sb[:], 1.0)
        nc.vector.memset(oner_sb[:], 1.0)
        nc.vector.memset(c256_sb[:], 1.0 / MSCALE)

        # ---- shared PSUM tiles; groups use disjoint regions ----
        ps_xe = ppool.tile([128, 8], FP32, tag="xe")        # xst cols
        ps_wqh = ppool.tile([128, H, BL], FP32, tag="wqh")
        ps_summ = ppool.tile([128, TC, BL, H], FP32, tag="summ")
        ps_xbtu = ppool.tile([128, BL, H], FP32, tag="xbtu")
        ps_rb = ppool.tile([128, NG, 2 * GH], FP32, tag="rb")  # recB | esumRow
        ps_out = ppool.tile([128, BL], FP32, tag="outp")

        def front(g):
            """xst -> wqh -> summary(+bias) -> exp for group g."""
            b0, b1 = g * GB, (g + 1) * GB
            for b in range(b0, b1):
                for c in range(TC):
                    jl = (b - b0) * TC + c
                    nc.tensor.matmul(ps_xe[:, b:b + 1], x_g[g][:, jl, :],
                                     ws_col[c],
                                     start=(c == 0), stop=(c == TC - 1))
            nc.vector.tensor_scalar_mul(xst_sb[:, b0:b1], ps_xe[:, b0:b1],
                                        c256_sb[:])
            for h in range(H):
                nc.tensor.matmul(ps_wqh[:, h, b0:b1],
                                 mt_sb[:, h * 128:(h + 1) * 128],
                                 xst_sb[:, b0:b1], start=True, stop=True)
            nc.vector.tensor_copy(wqh_sb[:, :, b0:b1], ps_wqh[:, :, b0:b1])
            for b in range(b0, b1):
                for c in range(TC):
                    jl = (b - b0) * TC + c
                    nc.tensor.matmul(ps_summ[:, c, b, :],
                                     xt_sb[g][:, jl, :], q0s[:, :],
                                     start=True, stop=False)
                    nc.tensor.matmul(ps_summ[:, c, b, :],
                                     xt_sb[g][:, jl, :], wqh_sb[:, :, b],
                                     start=False, stop=True)
            nc.scalar.activation(e_sb[:, :, b0:b1, :], ps_summ[:, :, b0:b1, :],
                                 AF.Exp)

        def mid(g):
            """esumRow -> 1/esum -> broadcast, and xbtu, then normalize."""
            b0, b1 = g * GB, (g + 1) * GB
            for c in range(TC):
                nc.tensor.matmul(
                    ps_rb[:1, g, GH:2 * GH], onec_sb[:],
                    e_sb[:, c, b0:b1, :].rearrange("t b h -> t (b h)"),
                    start=(c == 0), stop=(c == TC - 1))
            nc.vector.reciprocal(recr_sb[g][:], ps_rb[:1, g, GH:2 * GH])
            nc.tensor.matmul(ps_rb[:, g, :GH], oner_sb[:], recr_sb[g][:],
                             start=True, stop=True)
            for b in range(b0, b1):
                for c in range(TC):
                    jl = (b - b0) * TC + c
                    nc.tensor.matmul(ps_xbtu[:, b, :], x_g[g][:, jl, :],
                                     e_sb[:, c, b, :],
                                     start=(c == 0), stop=(c == TC - 1))
            nc.scalar.copy(xbtu_sb[:, b0:b1, :], ps_xbtu[:, b0:b1, :])
            nc.vector.tensor_mul(
                xbt_sb[:, b0:b1, :].rearrange("d b h -> d (b h)"),
                xbtu_sb[:, b0:b1, :].rearrange("d b h -> d (b h)"),
                ps_rb[:, g, :GH])

        def back(g):
            """output projection for group g."""
            b0, b1 = g * GB, (g + 1) * GB
            for h in range(H):
                nc.tensor.matmul(ps_out[:, b0:b1],
                                 nn_sb[:, h * 128:(h + 1) * 128],
                                 xbt_sb[:, b0:b1, h],
                                 start=(h == 0), stop=(h == H - 1))
            nc.scalar.copy(outt_sb[:, b0:b1], ps_out[:, b0:b1])

        front(0)
        front(1)
        mid(0)
        mid(1)
        back(0)
        back(1)

        # ---- y[dout, b] in one DMA; host transposes and adds b0 ----
        nc.sync.dma_start(y_d, outt_sb[:])


def _prep_in_maps(inputs):
    x = np.asarray(inputs["x"], dtype=np.float32)
    Wq = np.asarray(inputs["Wq"], dtype=np.float32)
    Wk = np.asarray(inputs["Wk"], dtype=np.float32)
    Wv = np.asarray(inputs["Wv"], dtype=np.float32)
    Wo = np.asarray(inputs["Wo"], dtype=np.float32)
    bk = np.asarray(inputs["bk"], dtype=np.float32)
    Ws = np.asarray(inputs["Ws"], dtype=np.float32).reshape(T)
    sws = float(Ws.sum())

    mt = np.zeros((128, H * 128), dtype=np.float32)
    nn = np.zeros((128, H * 128), dtype=np.float32)
    q0 = np.zeros((128, H), dtype=np.float32)
    for h in range(H):
        Wq_h = Wq[:, h * E:(h + 1) * E]
        Wk_h = Wk[:, h * E:(h + 1) * E]
        Wv_h = Wv[:, h * E:(h + 1) * E]
        Wo_h = Wo[h * E:(h + 1) * E, :]
        q0[:, h] = sws * SCALE * (Wq_h @ bk[h * E:(h + 1) * E])
        mt[:, h * 128:(h + 1) * 128] = (MSCALE * SCALE) * (Wk_h @ Wq_h.T)
        nn[:, h * 128:(h + 1) * 128] = Wv_h @ Wo_h

    shared = {"mt": mt.astype(NP_FP8), "nn": nn.astype(NP_BF16)}
    in_maps = []
    for core in range(N_CORES):
        xc = x[core * BL:(core + 1) * BL]                  # [BL, T, D]
        xq = xc.reshape(BL, TC, 128, D)
        xr = xq.transpose(2, 0, 1, 3).reshape(128, NJ * D)   # [t, (b c d)]
        xtr = xq.transpose(3, 0, 1, 2).reshape(128, NJ * 128)  # [d, (b c t)]
        x0 = np.concatenate(
            [xr[:, :XW],
             Ws.reshape(TC, 128).T.astype(np.float32),
             q0], axis=1)
        in_maps.append({
            "x0": np.ascontiguousarray(x0.astype(NP_BF16)),
            "x1": np.ascontiguousarray(xr[:, XW:].astype(NP_BF16)),
            "xt": np.ascontiguousarray(xtr.astype(NP_FP8)).reshape(
                128, NJ, 128),
            **shared,
        })
    return in_maps


def kernel(**inputs):
    if "nc" not in _cached:
        _cached["nc"] = _build_program()
    nc = _cached["nc"]
    in_maps = _prep_in_maps(inputs)
    res = run_bass_kernel_spmd(nc, in_maps, list(range(N_CORES)))
    _cached["last_results"] = res

    Wo = np.asarray(inputs["Wo"], dtype=np.float32)
    bv = np.asarray(inputs["bv"], dtype=np.float32)
    bo = np.asarray(inputs["bo"], dtype=np.float32)
    b0 = bo + bv @ Wo
    return np.concatenate(
        [res.results[c]["y"].T + b0 for c in range(N_CORES)], axis=0
    ).astype(np.float32)


# revision 21
# speedup vs baseline: 1.7400x; 1.0338x over previous
"""Temporal attention kernel for Trainium2, data-parallel over batch on 8 cores.

Reference computation (B=64, T=256, D=128, H=8, E=128):
    Q = x@Wq + bq; K = x@Wk + bk; V = x@Wv + bv          [B,T,H,E]
    scores  = einsum('bthd,bjhd->bhtj', Q, K)            [B,H,T,T]
    summary = (scale*scores) @ Ws + bs                   [B,H,T,1]
    beta    = softmax(summary, axis=t)                   [B,H,T]
    result  = sum_t V[b,t,h,:] * beta[b,h,t]             [B,H,E]
    out     = result.reshape(B,H*E) @ Wo + bo            [B,D]

Algebraic restructure (exact up to fp reassociation):
  * Ws contracts the key axis immediately, so K enters only through
    xs_b = x_b^T Ws, and the logits are
      summary[t,h] = x_b[t,:] @ (M_h xs_b + sws*q0_h)
    with M_h = scale*Wq_h Wk_h^T and q0_h = scale*Wq_h bk_h folded on host.
  * softmax over t is shift-invariant => bq/bs terms drop; logits are O(0.05)
    so exp() without max-subtraction is exact.
  * V and the output projection collapse: out = sum_h N_h^T xbt_h + b0 with
    N_h = Wv_h Wo_h and b0 = bo + Wo^T bv folded on host (b0 added on host).
  * beta normalization: 1/sum(exp) is broadcast down all 128 partitions with
    a rank-1 ones x recRow matmul, so the weighted V-sum is normalized by a
    single elementwise multiply - no transposes anywhere in the kernel.
  Precision: V path (x, N) travels bf16; logits-only tensors (xT, M) travel
  fp8e4m3 (M pre-scaled by 256, compensated in xst) - logits are O(0.05) and
  softmax is shift-tolerant, keeping rel err ~4e-3 << 2e-2 tol.
  x ships in both [t,.,d] (bf16) and [d,.,t] (fp8) layouts so no on-chip
  transposes of x are needed; DMAs are spread over the SP/Pool/Act queues.
"""

import contextlib

import numpy as np

import concourse.bacc as bacc
import concourse.bass as bass
import concourse.mybir as mybir
import concourse.tile as tile
from concourse.bass_utils import run_bass_kernel_spmd

N_CORES = 8
B, T, D = 64, 256, 128
H, E = 8, 128
HE = H * E
BL = B // N_CORES          # samples per core (8)
TC = T // 128              # 128-token chunks per sample (2)
NJ = BL * TC               # token chunks per core (16)
NG = 2                     # sample groups per core (pipelining)
GB = BL // NG              # samples per group (4)
HJ = NJ // NG              # chunks per group (8)
GH = GB * H                # (b,h) pairs per group (32)
SCALE = 1.0 / float(np.sqrt(np.float32(E)))
MSCALE = 256.0             # fp8 underflow guard on M, undone in xst

FP32 = mybir.dt.float32
BF16 = mybir.dt.bfloat16
FP8 = mybir.dt.float8e4
NP_BF16 = mybir.dt.np(BF16)
NP_FP8 = mybir.dt.np(FP8)
AF = mybir.ActivationFunctionType

XW = HJ * 128              # x cols per group
C_WS = XW                  # Ws chunks (TC cols) appended to x group 0
C_Q0 = C_WS + TC           # sws*scale*Wq_h@bk_h, H cols

_cached = {}


def _build_program():
    nc = bacc.Bacc("TRN2", target_bir_lowering=False, debug=False)

    x0_d = nc.dram_tensor("x0", [128, XW + TC + H], BF16,
                          kind="ExternalInput").ap()
    x1_d = nc.dram_tensor("x1", [128, XW], BF16, kind="ExternalInput").ap()
    xt_d = nc.dram_tensor("xt", [128, NJ, 128], FP8, kind="ExternalInput").ap()
    mt_d = nc.dram_tensor("mt", [128, H * 128], FP8, kind="ExternalInput").ap()
    nn_d = nc.dram_tensor("nn", [128, H * 128], BF16, kind="ExternalInput").ap()
    y_d = nc.dram_tensor("y", [D, BL], FP32, kind="ExternalOutput").ap()

    with tile.TileContext(nc) as tc:
        _emit(tc, x0_d, x1_d, xt_d, mt_d, nn_d, y_d)
    nc.compile()
    return nc


def _emit(tc, x0_d, x1_d, xt_d, mt_d, nn_d, y_d):
    nc = tc.nc
    with contextlib.ExitStack() as ctx:
        cpool = ctx.enter_context(tc.tile_pool(name="consts", bufs=1))
        ppool = ctx.enter_context(tc.tile_pool(name="psums", bufs=1,
                                               space="PSUM"))

        # ---- persistent SBUF tiles ----
        x0_sb = cpool.tile([128, XW + TC + H], BF16, tag="x0")
        x1_sb = cpool.tile([128, XW], BF16, tag="x1")
        xt_sb = [cpool.tile([128, HJ, 128], FP8, tag=f"xt{g}", name=f"xt{g}")
                 for g in range(NG)]
        mt_sb = cpool.tile([128, H * 128], FP8, tag="mt")
        nn_sb = cpool.tile([128, H * 128], BF16, tag="nn")
        onec_sb = cpool.tile([128, 1], BF16, tag="onec")    # ones column
        oner_sb = cpool.tile([1, 128], FP32, tag="oner")    # ones row
        c256_sb = cpool.tile([128, 1], FP32, tag="c256")    # 1/MSCALE
        xst_sb = cpool.tile([128, BL], BF16, tag="xst")     # [d, b] (/MSCALE)
        wqh_sb = cpool.tile([128, H, BL], BF16, tag="wqh")  # [d, h, b]
        e_sb = cpool.tile([128, TC, BL, H], BF16, tag="esb")  # [t, c, b, h]
        recr_sb = [cpool.tile([1, GH], FP32, tag=f"rr{g}", name=f"rr{g}")
                   for g in range(NG)]                      # 1/esum row
        xbtu_sb = cpool.tile([128, BL, H], BF16, tag="xbtu")  # unnormalized
        xbt_sb = cpool.tile([128, BL, H], BF16, tag="xbt")  # normalized
        outt_sb = cpool.tile([128, BL], FP32, tag="outt")   # [dout, b]

        x_g = [x0_sb[:, :XW].rearrange("t (j d) -> t j d", d=128),
               x1_sb.rearrange("t (j d) -> t j d", d=128)]
        ws_col = [x0_sb[:, C_WS + c:C_WS + c + 1] for c in range(TC)]
        q0s = x0_sb[:, C_Q0:C_Q0 + H]

        # ---- input DMAs across the three DMA-capable queues ----
        # SP: x0(+ws,q0), MT;  Pool: x1, xT0, xT1;  Act: (act table), NN
        nc.sync.dma_start(x0_sb[:], x0_d)
        nc.sync.dma_start(mt_sb[:], mt_d)
        nc.gpsimd.dma_start(x1_sb[:], x1_d)
        nc.gpsimd.dma_start(xt_sb[0][:], xt_d[:, :HJ, :])
        nc.gpsimd.dma_start(xt_sb[1][:], xt_d[:, HJ:, :])
        nc.scalar.dma_start(nn_sb[:], nn_d)

        # constants built on idle DVE lanes at t~0
        nc.vector.memset(onec_sb[:], 1.0)
        nc.vector.memset(oner_sb[:], 1.0)
        nc.vector.memset(c256_sb[:], 1.0 / MSCALE)

        # ---- shared PSUM tiles; groups use disjoint regions ----
        ps_xe = ppool.tile([128, 8], FP32, tag="xe")        # xst cols
        ps_wqh = ppool.tile([128, H, BL], FP32, tag="wqh")
        ps_summ = ppool.tile([128, TC, BL, H], FP32, tag="summ")
        ps_xbtu = ppool.tile([128, BL, H], FP32, tag="xbtu")
        ps_rb = ppool.tile([128, NG, 2 * GH], FP32, tag="rb")  # recB | esumRow
        ps_out = ppool.tile([128, BL], FP32, tag="outp")

        def xst(g):
            b0, b1 = g * GB, (g + 1) * GB
            for b in range(b0, b1):
                for c in range(TC):
                    jl = (b - b0) * TC + c
                    nc.tensor.matmul(ps_xe[:, b:b + 1], x_g[g][:, jl, :],
                                     ws_col[c],
                                     start=(c == 0), stop=(c == TC - 1))
            nc.vector.tensor_scalar_mul(xst_sb[:, b0:b1], ps_xe[:, b0:b1],
                                        c256_sb[:])

        def wqh(g):
            b0, b1 = g * GB, (g + 1) * GB
            for h in range(H):
                nc.tensor.matmul(ps_wqh[:, h, b0:b1],
                                 mt_sb[:, h * 128:(h + 1) * 128],
                                 xst_sb[:, b0:b1], start=True, stop=True)
            nc.vector.tensor_copy(wqh_sb[:, :, b0:b1], ps_wqh[:, :, b0:b1])

        def summ(g):
            b0, b1 = g * GB, (g + 1) * GB
            for b in range(b0, b1):
                for c in range(TC):
                    jl = (b - b0) * TC + c
                    nc.tensor.matmul(ps_summ[:, c, b, :],
                                     xt_sb[g][:, jl, :], q0s[:, :],
                                     start=True, stop=False)
                    nc.tensor.matmul(ps_summ[:, c, b, :],
                                     xt_sb[g][:, jl, :], wqh_sb[:, :, b],
                                     start=False, stop=True)
            nc.scalar.activation(e_sb[:, :, b0:b1, :], ps_summ[:, :, b0:b1, :],
                                 AF.Exp)

        def esum(g):
            b0, b1 = g * GB, (g + 1) * GB
            for c in range(TC):
                nc.tensor.matmul(
                    ps_rb[:1, g, GH:2 * GH], onec_sb[:],
                    e_sb[:, c, b0:b1, :].rearrange("t b h -> t (b h)"),
                    start=(c == 0), stop=(c == TC - 1))
            nc.vector.reciprocal(recr_sb[g][:], ps_rb[:1, g, GH:2 * GH])

        def xbtu(g):
            b0, b1 = g * GB, (g + 1) * GB
            for b in range(b0, b1):
                for c in range(TC):
                    jl = (b - b0) * TC + c
                    nc.tensor.matmul(ps_xbtu[:, b, :], x_g[g][:, jl, :],
                                     e_sb[:, c, b, :],
                                     start=(c == 0), stop=(c == TC - 1))
            nc.vector.tensor_copy(xbtu_sb[:, b0:b1, :], ps_xbtu[:, b0:b1, :])

        def recb(g):
            b0, b1 = g * GB, (g + 1) * GB
            nc.tensor.matmul(ps_rb[:, g, :GH], oner_sb[:], recr_sb[g][:],
                             start=True, stop=True)
            nc.vector.tensor_mul(
                xbt_sb[:, b0:b1, :].rearrange("d b h -> d (b h)"),
                xbtu_sb[:, b0:b1, :].rearrange("d b h -> d (b h)"),
                ps_rb[:, g, :GH])

        def back(g):
            b0, b1 = g * GB, (g + 1) * GB
            for h in range(H):
                nc.tensor.matmul(ps_out[:, b0:b1],
                                 nn_sb[:, h * 128:(h + 1) * 128],
                                 xbt_sb[:, b0:b1, h],
                                 start=(h == 0), stop=(h == H - 1))
            nc.scalar.copy(outt_sb[:, b0:b1], ps_out[:, b0:b1])

        # emission = per-engine program order; staged so no group's op
        # head-of-line-blocks the other group's earlier stage
        xst(0)
        xst(1)
        wqh(0)
        wqh(1)
        summ(0)
        summ(1)
        esum(0)
        xbtu(0)
        esum(1)
        xbtu(1)
        recb(0)
        recb(1)
        back(0)
        back(1)

        # ---- y[dout, b] in one DMA; host transposes and adds b0 ----
        nc.sync.dma_start(y_d, outt_sb[:])


def _prep_in_maps(inputs):
    x = np.asarray(inputs["x"], dtype=np.float32)
    Wq = np.asarray(inputs["Wq"], dtype=np.float32)
    Wk = np.asarray(inputs["Wk"], dtype=np.float32)
    Wv = np.asarray(inputs["Wv"], dtype=np.float32)
    Wo = np.asarray(inputs["Wo"], dtype=np.float32)
    bk = np.asarray(inputs["bk"], dtype=np.float32)
    Ws = np.asarray(inputs["Ws"], dtype=np.float32).reshape(T)
    sws = float(Ws.sum())

    mt = np.zeros((128, H * 128), dtype=np.float32)
    nn = np.zeros((128, H * 128), dtype=np.float32)
    q0 = np.zeros((128, H), dtype=np.float32)
    for h in range(H):
        Wq_h = Wq[:, h * E:(h + 1) * E]
        Wk_h = Wk[:, h * E:(h + 1) * E]
        Wv_h = Wv[:, h * E:(h + 1) * E]
        Wo_h = Wo[h * E:(h + 1) * E, :]
        q0[:, h] = sws * SCALE * (Wq_h @ bk[h * E:(h + 1) * E])
        mt[:, h * 128:(h + 1) * 128] = (MSCALE * SCALE) * (Wk_h @ Wq_h.T)
        nn[:, h * 128:(h + 1) * 128] = Wv_h @ Wo_h

    shared = {"mt": mt.astype(NP_FP8), "nn": nn.astype(NP_BF16)}
    in_maps = []
    for core in range(N_CORES):
        xc = x[core * BL:(core + 1) * BL]                  # [BL, T, D]
        xq = xc.reshape(BL, TC, 128, D)
        xr = xq.transpose(2, 0, 1, 3).reshape(128, NJ * D)   # [t, (b c d)]
        xtr = xq.transpose(3, 0, 1, 2).reshape(128, NJ * 128)  # [d, (b c t)]
        x0 = np.concatenate(
            [xr[:, :XW],
             Ws.reshape(TC, 128).T.astype(np.float32),
             q0], axis=1)
        in_maps.append({
            "x0": np.ascontiguousarray(x0.astype(NP_BF16)),
            "x1": np.ascontiguousarray(xr[:, XW:].astype(NP_BF16)),
            "xt": np.ascontiguousarray(xtr.astype(NP_FP8)).reshape(
                128, NJ, 128),
            **shared,
        })
    return in_maps


def kernel(**inputs):
    if "nc" not in _cached:
        _cached["nc"] = _build_program()
    nc = _cached["nc"]
    in_maps = _prep_in_maps(inputs)
    res = run_bass_kernel_spmd(nc, in_maps, list(range(N_CORES)))
    _cached["last_results"] = res

    Wo = np.asarray(inputs["Wo"], dtype=np.float32)
    bv = np.asarray(inputs["bv"], dtype=np.float32)
    bo = np.asarray(inputs["bo"], dtype=np.float32)
    b0 = bo + bv @ Wo
    return np.concatenate(
        [res.results[c]["y"].T + b0 for c in range(N_CORES)], axis=0
    ).astype(np.float32)
